# revision 11
# baseline (speedup 1.0000x reference)
"""Trainium2 Bass kernel for CLIP-style symmetric contrastive loss.

Problem: image_features [8192, 1024] f32, text_features [8192, 1024] f32.
  loss = 0.5 * (CE(logits, diag) + CE(logits.T, diag)),
  logits = cosine_similarity(img, txt) / 0.07.

Distribution: shard image rows across 8 NeuronCores. Each core m computes the
slab S_m = img_n[m] @ txt_n.T  ([1024, 8192] cosines) against the full
normalized text matrix in fp8 (DoubleRow perf-mode matmuls, 2x PE rate),
reduces exp(C*S - C + LOG_OFF) along rows (ACT accum) and columns (fp8
DoubleRow ones-matmuls), and one [N+2]-float AllReduce combines column sums +
per-core scalars. A dummy 16-float AllReduce at kernel start absorbs the CC
ring init (~50 us) so only ~10 us of collective sits on the critical path.

The text matrix ships pre-transposed ([D, N]) and pre-quantized to fp8e4m3 by
the host; its normalization (column rsqrt scale) happens on-device per
512-column chunk, pipelined with the main matmul of the previous chunk.
Elementwise work (squares, normalize scaling) is split across Vector and
GpSimd so the PE never starves (it downclocks 2x when idle >ramp).

Math (C = 1/T bounds every logit; LOG_OFF keeps exp outputs ~O(1) for fp8):
  loss = (C - LOG_OFF) + (R + L - 2C * Draw) / (2N)
    R    = sum_i log sum_j exp(C*s_ij - C + LOG_OFF)
    L    = same over columns
    Draw = sum_i cos(img_i, txt_i)
"""
import math
import threading
from contextlib import ExitStack

import ml_dtypes
import numpy as np

import concourse.bacc as bacc
import concourse.bass as bass
import concourse.bass_isa as bass_isa
import concourse.mybir as mybir
import concourse.tile as tile
from concourse.bass_utils import run_bass_kernel_spmd

F32 = mybir.dt.float32
BF16 = mybir.dt.bfloat16
FP8 = mybir.dt.float8e4
AF = mybir.ActivationFunctionType
ALU = mybir.AluOpType
DR = mybir.MatmulPerfMode.DoubleRow

N_CORES = 8
N = 8192
D = 1024
TEMPERATURE = 0.07


def build_nc(n=N, d=D, n_cores=N_CORES, no_collective=False):
    nc = bacc.Bacc("TRN2", target_bir_lowering=False, debug=False,
                   num_devices=n_cores)
    rows = n // n_cores
    imgT = nc.dram_tensor("imgT", [d, rows], BF16, kind="ExternalInput").ap()
    totT = nc.dram_tensor("totT", [d, rows], BF16, kind="ExternalInput").ap()
    txt8 = nc.dram_tensor("txt8", [d, n], FP8, kind="ExternalInput").ap()
    ones8 = nc.dram_tensor("ones8", [128, 256], FP8, kind="ExternalInput").ap()
    onesb = nc.dram_tensor("onesb", [128, 128], BF16, kind="ExternalInput").ap()
    out = nc.dram_tensor("out", [1, 1], F32, kind="ExternalOutput").ap()

    with tile.TileContext(nc) as tc:
        _body(tc, imgT, totT, txt8, ones8, onesb, out,
              n=n, d=d, rows=rows, n_cores=n_cores,
              no_collective=no_collective)
    nc.compile()
    return nc


def _body(tc, imgT, totT, txt8, ones8, onesb, out, *, n, d, rows, n_cores,
          no_collective):
    nc = tc.nc
    P = 128
    kt = d // P                      # 8 contraction tiles
    kp = kt // 2                     # 4 DoubleRow k-pairs
    CH = 512
    n_ch = n // CH                   # 16 column chunks
    rp = rows // P                   # 8 image row blocks
    rc_ch = rows // CH               # 2 phase-A column chunks
    inv_t = float(1.0 / TEMPERATURE)
    log_off = float(20.0 * math.log(2.0))
    ebias_v = float(log_off - inv_t)         # exp bias: -C + LOG_OFF
    W = n + 2                                # collective payload floats

    with ExitStack() as ctx:
        persist = ctx.enter_context(tc.tile_pool(name="persist", bufs=1))
        sq8p = ctx.enter_context(tc.tile_pool(name="sq8p", bufs=3))
        rsp = ctx.enter_context(tc.tile_pool(name="rsp", bufs=2))
        exsp = ctx.enter_context(tc.tile_pool(name="exsp", bufs=2))
        v1 = ctx.enter_context(tc.tile_pool(name="v1", bufs=1))
        psum_mm = ctx.enter_context(tc.tile_pool(name="psum_mm", bufs=4,
                                                 space="PSUM"))
        psum_n = ctx.enter_context(tc.tile_pool(name="psum_n", bufs=2,
                                                space="PSUM"))
        dram = ctx.enter_context(tc.tile_pool(name="dram", bufs=1,
                                              space="DRAM"))

        # chunk-major: [P, chunk, k, col] so per-chunk slices are contiguous
        txt8s = persist.tile([P, n_ch, kt, CH], FP8, tag="txt8s")   # 8 MB
        img8 = persist.tile([P, rc_ch, kt, CH], FP8, tag="img8")    # 1 MB
        imgT_sb = persist.tile([P, rc_ch, kt, CH], BF16, tag="imgT")
        totT_sb = persist.tile([P, rc_ch, kt, CH], BF16, tag="totT")
        rs_i = persist.tile([P, rows], BF16, tag="rs_i")
        rs_to = persist.tile([P, rows], BF16, tag="rs_to")
        ddv = persist.tile([P, rows], F32, tag="ddv")
        rparts = persist.tile([P, rp * n_ch], F32, tag="rparts")
        vecs = persist.tile([P, 16], F32, tag="vecs")
        ones8_sb = persist.tile([P, 2, P], FP8, tag="ones8")
        onesb_sb = persist.tile([P, P], BF16, tag="onesb")
        ebias = persist.tile([P, 1], F32, tag="ebias")
        cs_sb = persist.tile([P, n // P], F32, tag="cs_sb")
        ln_cs = persist.tile([P, n // P], F32, tag="ln_cs")

        warm = dram.tile([1, 16], F32, tag="warm")
        warm_out = dram.tile([1, 16], F32, tag="warm_out", addr_space="Shared")
        cbuf = dram.tile([1, W], F32, tag="cbuf")
        cbuf_out = dram.tile([1, W], F32, tag="cbuf_out", addr_space="Shared")

        grp = [list(range(n_cores))]

        # --- warm-up dummy collective: absorbs CC init + core start skew ----
        wsb = v1.tile([1, 16], F32, tag="wsb")
        nc.gpsimd.memset(wsb[:], 0.0)
        nc.sync.dma_start(warm[:], wsb[:])
        if not no_collective:
            nc.gpsimd.collective_compute(
                "AllReduce", ALU.add, replica_groups=grp,
                ins=[warm[:].opt()], outs=[warm_out[:].opt()])

        # --- constants + input DMAs ----------------------------------------
        nc.sync.dma_start(ones8_sb[:, 0, :], ones8[:, 0:P])
        nc.sync.dma_start(ones8_sb[:, 1, :], ones8[:, P:2 * P])
        nc.sync.dma_start(onesb_sb[:], onesb[:])
        nc.gpsimd.memset(ebias[:], ebias_v)

        for rc in range(rc_ch):
            csl = slice(rc * CH, (rc + 1) * CH)
            for k in range(kt):
                nc.sync.dma_start(imgT_sb[:, rc, k, :],
                                  imgT[k * P:(k + 1) * P, csl])
        for c in range(2):               # first text chunks early
            csl = slice(c * CH, (c + 1) * CH)
            for k in range(kt):
                nc.sync.dma_start(txt8s[:, c, k, :],
                                  txt8[k * P:(k + 1) * P, csl])
        for rc in range(rc_ch):
            csl = slice(rc * CH, (rc + 1) * CH)
            for k in range(kt):
                nc.sync.dma_start(totT_sb[:, rc, k, :],
                                  totT[k * P:(k + 1) * P, csl])
        for c in range(2, n_ch):
            csl = slice(c * CH, (c + 1) * CH)
            for k in range(kt):
                nc.sync.dma_start(txt8s[:, c, k, :],
                                  txt8[k * P:(k + 1) * P, csl])

        def col_rsqrt(src_ap, dst_ap, eng):
            """dst_ap [P, CH] (bf16, partition-replicated) =
            1/sqrt(colsum(src_ap^2)) for a [P, kt, CH] source slice."""
            sq = sq8p.tile([P, kt, CH], FP8, tag="sq8")
            eng.tensor_tensor(sq[:], src_ap, src_ap, ALU.mult)
            ps = psum_n.tile([P, CH], F32, tag="nps")
            for t in range(kp):
                nc.tensor.matmul(ps[:], ones8_sb[:],
                                 sq[:, 2 * t:2 * t + 2, :],
                                 start=(t == 0), stop=(t == kp - 1),
                                 perf_mode=DR)
            r32 = rsp.tile([P, CH], F32, tag="r32")
            nc.vector.reciprocal_approx_fast(r32[:], ps[:])
            nc.scalar.activation(dst_ap, r32[:], AF.Sqrt)

        # --- Phase A: image (+ own-text) norms, quantize, diag dots --------
        for rc in range(rc_ch):
            sl = slice(rc * CH, (rc + 1) * CH)
            col_rsqrt(imgT_sb[:, rc, :, :], rs_i[:, sl], nc.vector)
            nc.vector.tensor_tensor(
                img8[:, rc, :, :], imgT_sb[:, rc, :, :],
                rs_i[:, sl].unsqueeze(1).broadcast_to((P, kt, CH)), ALU.mult)
            col_rsqrt(totT_sb[:, rc, :, :], rs_to[:, sl], nc.gpsimd)
            prod = exsp.tile([P, kt, CH], BF16, tag="prod", bufs=1)
            nc.gpsimd.tensor_tensor(prod[:], imgT_sb[:, rc, :, :],
                                    totT_sb[:, rc, :, :], ALU.mult)
            dps = psum_n.tile([P, CH], F32, tag="cps", bufs=1)
            for k in range(kt):
                nc.tensor.matmul(dps[:], onesb_sb[:], prod[:, k, :],
                                 start=(k == 0), stop=(k == kt - 1))
            nc.vector.tensor_copy(ddv[:, sl], dps[:])

        # --- Phases B+C interleaved: per-chunk text prep + main matmul -----
        cs_pend = {}

        def emit_colsum(c):
            exs, csl = cs_pend.pop(c)
            cps = psum_n.tile([P, CH], F32, tag="cps", bufs=1)
            for u in range(rp // 2):
                nc.tensor.matmul(cps[:], ones8_sb[:],
                                 exs[:, 2 * u:2 * u + 2, :],
                                 start=(u == 0), stop=(u == rp // 2 - 1),
                                 perf_mode=DR)
            csr = rsp.tile([1, CH], F32, tag="csr")
            nc.vector.tensor_copy(csr[:], cps[0:1, :])
            nc.sync.dma_start(cbuf[0:1, csl], csr[:])

        for c in range(n_ch):
            csl = slice(c * CH, (c + 1) * CH)
            # text chunk normalize (in place, fp8); alternate square engine
            rst = rsp.tile([P, CH], BF16, tag="rst")
            eng = nc.vector if c % 2 == 0 else nc.gpsimd
            oeng = nc.gpsimd if c % 2 == 0 else nc.vector
            col_rsqrt(txt8s[:, c, :, :], rst[:], eng)
            oeng.tensor_tensor(
                txt8s[:, c, :, :], txt8s[:, c, :, :],
                rst[:].unsqueeze(1).broadcast_to((P, kt, CH)), ALU.mult)

            exs = exsp.tile([P, rp, CH], FP8, tag="exs")
            for p in range(rp):
                if p == rp // 2 and (c - 1) in cs_pend:
                    emit_colsum(c - 1)   # mid-chunk: deps long resolved
                rc, pp = divmod(p, rp // rc_ch)
                mm = psum_mm.tile([P, CH], F32, tag="mm")
                for t in range(kp):
                    nc.tensor.matmul(
                        mm[:],
                        img8[:, rc, 2 * t:2 * t + 2, pp * P:(pp + 1) * P],
                        txt8s[:, c, 2 * t:2 * t + 2, :],
                        start=(t == 0), stop=(t == kp - 1), perf_mode=DR)
                nc.scalar.activation(
                    exs[:, p, :], mm[:], AF.Exp, bias=ebias[:, 0:1],
                    scale=inv_t,
                    accum_out=rparts[:, p * n_ch + c:p * n_ch + c + 1])
            cs_pend[c] = (exs, csl)
        emit_colsum(n_ch - 1)

        # --- local scalars --------------------------------------------------
        # R partial: rowsum over chunks, ln, sum over (q, p), partition-reduce
        rsum = v1.tile([P, rp], F32, tag="rsum")
        for p in range(rp):
            nc.vector.tensor_reduce(rsum[:, p:p + 1],
                                    rparts[:, p * n_ch:(p + 1) * n_ch],
                                    axis=mybir.AxisListType.X, op=ALU.add)
        lnr = v1.tile([P, rp], F32, tag="lnr")
        nc.scalar.activation(lnr[:], rsum[:], AF.Ln)
        nc.vector.tensor_reduce(vecs[:, 0:1], lnr[:],
                                axis=mybir.AxisListType.X, op=ALU.add)
        nc.gpsimd.partition_all_reduce(vecs[:, 1:2], vecs[:, 0:1], channels=P,
                                       reduce_op=bass_isa.ReduceOp.add)
        # Draw partial: diag cosines (replicated over partitions)
        nc.vector.tensor_tensor(ddv[:], ddv[:], rs_i[:], ALU.mult)
        nc.vector.tensor_tensor(ddv[:], ddv[:], rs_to[:], ALU.mult)
        nc.vector.tensor_reduce(vecs[:, 2:3], ddv[:],
                                axis=mybir.AxisListType.X, op=ALU.add)

        nc.sync.dma_start(cbuf[0:1, n:n + 1], vecs[0:1, 1:2])
        nc.sync.dma_start(cbuf[0:1, n + 1:n + 2], vecs[0:1, 2:3])

        # --- AllReduce + finish ---------------------------------------------
        if no_collective:
            nc.sync.dma_start(cbuf_out[:], cbuf[:])
        else:
            nc.gpsimd.collective_compute(
                "AllReduce", ALU.add, replica_groups=grp,
                ins=[cbuf[:].opt()], outs=[cbuf_out[:].opt()])

        nc.sync.dma_start(
            cs_sb[:], cbuf_out[0:1, 0:n].rearrange("a (p x) -> (a p) x", p=P))
        nc.scalar.activation(ln_cs[:], cs_sb[:], AF.Ln)
        nc.vector.tensor_reduce(vecs[:, 3:4], ln_cs[:],
                                axis=mybir.AxisListType.X, op=ALU.add)
        nc.gpsimd.partition_all_reduce(vecs[:, 4:5], vecs[:, 3:4], channels=P,
                                       reduce_op=bass_isa.ReduceOp.add)
        rd = v1.tile([P, 8], F32, tag="rd")
        nc.sync.dma_start(rd[0:1, 0:2], cbuf_out[0:1, n:n + 2])

        # loss = (C - LOG_OFF) + (R + L - 2C*Draw) / (2N)
        fin = v1.tile([P, 8], F32, tag="fin")
        nc.vector.tensor_tensor(fin[0:1, 0:1], rd[0:1, 0:1], vecs[0:1, 4:5],
                                ALU.add)                        # R + L
        nc.vector.tensor_scalar_mul(fin[0:1, 1:2], rd[0:1, 1:2],
                                    float(-2.0 * inv_t))        # -2C*Draw
        nc.vector.tensor_tensor(fin[0:1, 2:3], fin[0:1, 0:1], fin[0:1, 1:2],
                                ALU.add)
        nc.scalar.activation(fin[0:1, 3:4], fin[0:1, 2:3], AF.Copy,
                             bias=float(inv_t - log_off),
                             scale=float(1.0 / (2 * n)))
        nc.sync.dma_start(out[0:1, 0:1], fin[0:1, 3:4])


def make_in_maps(image_features, text_features, n=N, d=D, n_cores=N_CORES):
    image_features = np.asarray(image_features, dtype=np.float32)
    text_features = np.asarray(text_features, dtype=np.float32)
    rows = n // n_cores
    txt8 = np.ascontiguousarray(text_features.T).astype(ml_dtypes.float8_e4m3)
    ones8 = np.ones((128, 256), dtype=ml_dtypes.float8_e4m3)
    onesb = np.ones((128, 128), dtype=ml_dtypes.bfloat16)
    maps = []
    for m in range(n_cores):
        sl = slice(m * rows, (m + 1) * rows)
        maps.append({
            "imgT": np.ascontiguousarray(
                image_features[sl].T).astype(ml_dtypes.bfloat16),
            "totT": np.ascontiguousarray(
                text_features[sl].T).astype(ml_dtypes.bfloat16),
            "txt8": txt8,
            "ones8": ones8,
            "onesb": onesb,
        })
    return maps


_CACHE = {}
_LOCK = threading.Lock()


def _get_nc():
    with _LOCK:
        if "nc" not in _CACHE:
            _CACHE["nc"] = build_nc()
        return _CACHE["nc"]


def kernel(image_features, text_features):
    image_features = np.asarray(image_features, dtype=np.float32)
    text_features = np.asarray(text_features, dtype=np.float32)
    assert image_features.shape == (N, D) and text_features.shape == (N, D)
    nc = _get_nc()
    in_maps = make_in_maps(image_features, text_features)
    res = run_bass_kernel_spmd(nc, in_maps, list(range(N_CORES)))
    val = np.float32(res.results[0]["out"][0, 0])
    return np.array(val, dtype=np.float32)


# revision 15
# speedup vs baseline: 1.3963x; 1.3963x over previous
"""Trainium2 Bass kernel for CLIP-style symmetric contrastive loss.

Problem: image_features [8192, 1024] f32, text_features [8192, 1024] f32.
  loss = 0.5 * (CE(logits, diag) + CE(logits.T, diag)),
  logits = cosine_similarity(img, txt) / 0.07.

Distribution: shard image rows across 8 NeuronCores. Each core computes its
[8192, 1024] slab of logits TRANSPOSED -- text columns on PSUM partitions,
image rows on the free axis -- via fp8 DoubleRow matmuls (2x PE rate) with
raw (unnormalized) text as the stationary operand. The per-text-column
1/||t_j|| then rides the ACT exp's per-partition *scale* operand, so the
8.4M-element text normalization multiply never happens. Column sums fall out
of the exp's accum_out; row sums are DoubleRow ones-matmuls accumulated in a
persistent PSUM bank pair across all 16 chunks. One [N+2]-float AllReduce
combines column sums + per-core scalars; a dummy AllReduce at kernel start
absorbs the ~50 us CC-ring init off the critical path.

Math (C = 1/T bounds every logit; LOG_OFF keeps exp outputs ~O(1) for fp8):
  loss = (C - LOG_OFF) + (R + L - 2C * Draw) / (2N)
    R    = sum_i log sum_j exp(C*s_ij - C + LOG_OFF)
    L    = same over columns
    Draw = sum_i cos(img_i, txt_i)
"""
import math
import threading
from contextlib import ExitStack

import ml_dtypes
import numpy as np

import concourse.bacc as bacc
import concourse.bass as bass
import concourse.bass_isa as bass_isa
import concourse.mybir as mybir
import concourse.tile as tile
from concourse.bass_utils import run_bass_kernel_spmd

F32 = mybir.dt.float32
BF16 = mybir.dt.bfloat16
FP8 = mybir.dt.float8e4
AF = mybir.ActivationFunctionType
ALU = mybir.AluOpType
DR = mybir.MatmulPerfMode.DoubleRow

N_CORES = 8
N = 8192
D = 1024
TEMPERATURE = 0.07


def build_nc(n=N, d=D, n_cores=N_CORES, no_collective=False):
    nc = bacc.Bacc("TRN2", target_bir_lowering=False, debug=False,
                   num_devices=n_cores)
    rows = n // n_cores
    P = 128
    kt = d // P
    CH = 512
    # all feature inputs ship pre-permuted to SBUF tile layout:
    # [chunk, partition(d%128... see make_in_maps), k, col]
    imgT = nc.dram_tensor("imgT", [rows // CH, P, kt, CH], BF16,
                          kind="ExternalInput").ap()
    totT = nc.dram_tensor("totT", [rows // CH, P, kt, CH], BF16,
                          kind="ExternalInput").ap()
    txt8 = nc.dram_tensor("txt8", [n // CH, P, kt, CH], FP8,
                          kind="ExternalInput").ap()
    ones8 = nc.dram_tensor("ones8", [128, 256], FP8, kind="ExternalInput").ap()
    onesb = nc.dram_tensor("onesb", [128, 128], BF16, kind="ExternalInput").ap()
    out = nc.dram_tensor("out", [1, 1], F32, kind="ExternalOutput").ap()

    with tile.TileContext(nc) as tc:
        _body(tc, imgT, totT, txt8, ones8, onesb, out,
              n=n, d=d, rows=rows, n_cores=n_cores,
              no_collective=no_collective)
    nc.compile()
    return nc


def _body(tc, imgT, totT, txt8, ones8, onesb, out, *, n, d, rows, n_cores,
          no_collective):
    nc = tc.nc
    P = 128
    kt = d // P                      # 8 contraction tiles
    kp = kt // 2                     # 4 DoubleRow k-pairs
    CH = 512
    n_ch = n // CH                   # 16 text column chunks
    jb_n = CH // P                   # 4 j-blocks per chunk
    rc_ch = rows // CH               # 2 image row chunks
    inv_t = float(1.0 / TEMPERATURE)
    log_off = float(20.0 * math.log(2.0))
    ebias_v = float(log_off - inv_t)         # exp bias: -C + LOG_OFF
    W = n + 2                                # collective payload floats

    with ExitStack() as ctx:
        persist = ctx.enter_context(tc.tile_pool(name="persist", bufs=1))
        sq8p = ctx.enter_context(tc.tile_pool(name="sq8p", bufs=3))
        rsp = ctx.enter_context(tc.tile_pool(name="rsp", bufs=2))
        exsp = ctx.enter_context(tc.tile_pool(name="exsp", bufs=2))
        v1 = ctx.enter_context(tc.tile_pool(name="v1", bufs=1))
        psum_mm = ctx.enter_context(tc.tile_pool(name="psum_mm", bufs=2,
                                                 space="PSUM"))
        psum_n = ctx.enter_context(tc.tile_pool(name="psum_n", bufs=1,
                                                space="PSUM"))
        dram = ctx.enter_context(tc.tile_pool(name="dram", bufs=1,
                                              space="DRAM"))

        txt8s = persist.tile([P, n_ch, kt, CH], FP8, tag="txt8s")   # 8 MB
        img8 = persist.tile([P, rc_ch, kt, CH], FP8, tag="img8")    # 1 MB
        imgT_sb = persist.tile([P, rc_ch, kt, CH], BF16, tag="imgT")
        totT_sb = persist.tile([P, rc_ch, kt, CH], BF16, tag="totT")
        rs_i = persist.tile([P, rows], BF16, tag="rs_i")
        rs_to = persist.tile([P, rows], BF16, tag="rs_to")
        ddv = persist.tile([P, rows], F32, tag="ddv")
        scl = persist.tile([P, n_ch * jb_n], F32, tag="scl")  # C/||t_j||
        cparts = persist.tile([P, n_ch * jb_n], F32, tag="cparts")
        vecs = persist.tile([P, 16], F32, tag="vecs")
        ones8_sb = persist.tile([P, 2, P], FP8, tag="ones8")
        onesb_sb = persist.tile([P, P], BF16, tag="onesb")
        ebias = persist.tile([P, 1], F32, tag="ebias")
        cs_sb = persist.tile([P, n // P], F32, tag="cs_sb")
        ln_cs = persist.tile([P, n // P], F32, tag="ln_cs")
        rps = psum_n.tile([P, rows], F32, tag="rps")  # rowsum accum, 2 banks

        warm = dram.tile([1, 16], F32, tag="warm")
        warm_out = dram.tile([1, 16], F32, tag="warm_out", addr_space="Shared")
        cbuf = dram.tile([1, W], F32, tag="cbuf")
        cbuf_out = dram.tile([1, W], F32, tag="cbuf_out", addr_space="Shared")

        grp = [list(range(n_cores))]

        # --- warm-up dummy collective: absorbs CC init + core start skew ----
        wsb = v1.tile([1, 16], F32, tag="wsb")
        nc.gpsimd.memset(wsb[:], 0.0)
        nc.sync.dma_start(warm[:], wsb[:])
        if not no_collective:
            nc.gpsimd.collective_compute(
                "AllReduce", ALU.add, replica_groups=grp,
                ins=[warm[:].opt()], outs=[warm_out[:].opt()])

        # --- constants + input DMAs (everything pre-permuted on host) -------
        nc.sync.dma_start(ones8_sb[:, 0, :], ones8[:, 0:P])
        nc.sync.dma_start(ones8_sb[:, 1, :], ones8[:, P:2 * P])
        nc.sync.dma_start(onesb_sb[:], onesb[:])
        nc.gpsimd.memset(ebias[:], ebias_v)

        for rc in range(rc_ch):
            nc.sync.dma_start(imgT_sb[:, rc, :, :], imgT[rc])
        for c in range(2):
            nc.sync.dma_start(txt8s[:, c, :, :], txt8[c])
        for rc in range(rc_ch):
            nc.sync.dma_start(totT_sb[:, rc, :, :], totT[rc])
        for c in range(2, n_ch):
            nc.sync.dma_start(txt8s[:, c, :, :], txt8[c])

        def col_ssq(src_ap, eng):
            """[P, CH] f32 PSUM (partition-replicated) = colsum(src^2)."""
            sq = sq8p.tile([P, kt, CH], FP8, tag="sq8")
            eng.tensor_tensor(sq[:], src_ap, src_ap, ALU.mult)
            ps = psum_n.tile([P, CH], F32, tag="nps", bufs=2)
            for t in range(kp):
                nc.tensor.matmul(ps[:], ones8_sb[:],
                                 sq[:, 2 * t:2 * t + 2, :],
                                 start=(t == 0), stop=(t == kp - 1),
                                 perf_mode=DR)
            return ps

        # --- Phase A: image (+ own-text) norms, quantize, diag dots --------
        for rc in range(rc_ch):
            sl = slice(rc * CH, (rc + 1) * CH)
            ps = col_ssq(imgT_sb[:, rc, :, :], nc.vector)
            r32 = rsp.tile([P, CH], F32, tag="r32")
            nc.vector.reciprocal_approx_fast(r32[:], ps[:])
            nc.scalar.activation(rs_i[:, sl], r32[:], AF.Sqrt)
            for k in range(kt):
                eng = nc.vector if k % 2 == 0 else nc.gpsimd
                eng.tensor_tensor(img8[:, rc, k, :], imgT_sb[:, rc, k, :],
                                  rs_i[:, sl], ALU.mult)
            ps2 = col_ssq(totT_sb[:, rc, :, :], nc.gpsimd)
            r32b = rsp.tile([P, CH], F32, tag="r32")
            nc.vector.reciprocal_approx_fast(r32b[:], ps2[:])
            nc.scalar.activation(rs_to[:, sl], r32b[:], AF.Sqrt)
            prod = exsp.tile([P, kt, CH], BF16, tag="prod", bufs=1)
            nc.gpsimd.tensor_tensor(prod[:], imgT_sb[:, rc, :, :],
                                    totT_sb[:, rc, :, :], ALU.mult)
            dps = psum_n.tile([P, CH], F32, tag="nps", bufs=2)
            for k in range(kt):
                nc.tensor.matmul(dps[:], onesb_sb[:], prod[:, k, :],
                                 start=(k == 0), stop=(k == kt - 1))
            nc.vector.tensor_copy(ddv[:, sl], dps[:])

        # --- main loop: per text chunk --------------------------------------
        for c in range(n_ch):
            # text column scales: C/||t_j||, landed j-on-partition via DMA
            eng = nc.vector if c % 2 == 0 else nc.gpsimd
            ps = col_ssq(txt8s[:, c, :, :], eng)
            r1 = rsp.tile([1, CH], F32, tag="r1")
            nc.vector.reciprocal_approx_fast(r1[:], ps[0:1, :])
            rs1 = rsp.tile([1, CH], F32, tag="rs1")
            nc.scalar.activation(rs1[:], r1[:], AF.Sqrt,
                                 scale=float(inv_t * inv_t))  # C * rsqrt
            nc.sync.dma_start(
                scl[:, c * jb_n:(c + 1) * jb_n],
                rs1[0:1, :].rearrange("a (x p) -> (a p) x", p=P))

            exs = exsp.tile([P, jb_n, rows], FP8, tag="exs")
            for jb in range(jb_n):
                col = c * jb_n + jb
                mm = psum_mm.tile([P, rows], F32, tag="mm")
                for rc in range(rc_ch):
                    for t in range(kp):
                        nc.tensor.matmul(
                            mm[:, rc * CH:(rc + 1) * CH],
                            txt8s[:, c, 2 * t:2 * t + 2, jb * P:(jb + 1) * P],
                            img8[:, rc, 2 * t:2 * t + 2, :],
                            start=(t == 0), stop=(t == kp - 1), perf_mode=DR)
                nc.scalar.activation(
                    exs[:, jb, :], mm[:], AF.Exp, bias=ebias[:, 0:1],
                    scale=scl[:, col:col + 1],
                    accum_out=cparts[:, col:col + 1])
            # rowsum partials: DoubleRow ones-matmuls into persistent PSUM
            for u in range(jb_n // 2):
                for h in range(rc_ch):
                    nc.tensor.matmul(
                        rps[:, h * CH:(h + 1) * CH], ones8_sb[:],
                        exs[:, 2 * u:2 * u + 2, h * CH:(h + 1) * CH],
                        start=(c == 0 and u == 0),
                        stop=(c == n_ch - 1 and u == jb_n // 2 - 1),
                        perf_mode=DR)

        # --- local scalars ----------------------------------------------------
        # R partial: rowsums sit replicated in rps
        lnr = v1.tile([1, rows], F32, tag="lnr")
        nc.scalar.activation(lnr[:], rps[0:1, :], AF.Ln)
        nc.vector.tensor_reduce(vecs[0:1, 1:2], lnr[:],
                                axis=mybir.AxisListType.X, op=ALU.add)
        # Draw partial: diag cosines (replicated over partitions)
        nc.vector.tensor_tensor(ddv[:], ddv[:], rs_i[:], ALU.mult)
        nc.vector.tensor_tensor(ddv[:], ddv[:], rs_to[:], ALU.mult)
        nc.vector.tensor_reduce(vecs[:, 2:3], ddv[:],
                                axis=mybir.AxisListType.X, op=ALU.add)

        # colsum partials land in cbuf q-major; the post-collective reader
        # uses the same permutation and only ever sums ln(colsum), so the
        # order inside the payload is irrelevant (identical on every core).
        nc.sync.dma_start(
            cbuf[0:1, 0:n].rearrange("a (p x) -> (a p) x", p=P), cparts[:])
        nc.sync.dma_start(cbuf[0:1, n:n + 1], vecs[0:1, 1:2])
        nc.sync.dma_start(cbuf[0:1, n + 1:n + 2], vecs[0:1, 2:3])

        # --- AllReduce + finish ---------------------------------------------
        if no_collective:
            nc.sync.dma_start(cbuf_out[:], cbuf[:])
        else:
            nc.gpsimd.collective_compute(
                "AllReduce", ALU.add, replica_groups=grp,
                ins=[cbuf[:].opt()], outs=[cbuf_out[:].opt()])

        nc.sync.dma_start(
            cs_sb[:], cbuf_out[0:1, 0:n].rearrange("a (p x) -> (a p) x", p=P))
        nc.scalar.activation(ln_cs[:], cs_sb[:], AF.Ln)
        nc.vector.tensor_reduce(vecs[:, 3:4], ln_cs[:],
                                axis=mybir.AxisListType.X, op=ALU.add)
        nc.gpsimd.partition_all_reduce(vecs[:, 4:5], vecs[:, 3:4], channels=P,
                                       reduce_op=bass_isa.ReduceOp.add)
        rd = v1.tile([P, 8], F32, tag="rd")
        nc.sync.dma_start(rd[0:1, 0:2], cbuf_out[0:1, n:n + 2])

        # loss = (C - LOG_OFF) + (R + L - 2C*Draw) / (2N)
        fin = v1.tile([P, 8], F32, tag="fin")
        nc.vector.tensor_tensor(fin[0:1, 0:1], rd[0:1, 0:1], vecs[0:1, 4:5],
                                ALU.add)                        # R + L
        nc.vector.tensor_scalar_mul(fin[0:1, 1:2], rd[0:1, 1:2],
                                    float(-2.0 * inv_t))        # -2C*Draw
        nc.vector.tensor_tensor(fin[0:1, 2:3], fin[0:1, 0:1], fin[0:1, 1:2],
                                ALU.add)
        nc.scalar.activation(fin[0:1, 3:4], fin[0:1, 2:3], AF.Copy,
                             bias=float(inv_t - log_off),
                             scale=float(1.0 / (2 * n)))
        nc.sync.dma_start(out[0:1, 0:1], fin[0:1, 3:4])


def _permute(xT, ch):
    """[d, cols] -> [cols//ch, 128, d//128, ch] (SBUF tile layout, dense)."""
    d, cols = xT.shape
    return np.ascontiguousarray(
        xT.reshape(d // 128, 128, cols // ch, ch).transpose(2, 1, 0, 3))


def make_in_maps(image_features, text_features, n=N, d=D, n_cores=N_CORES):
    image_features = np.asarray(image_features, dtype=np.float32)
    text_features = np.asarray(text_features, dtype=np.float32)
    rows = n // n_cores
    txt8 = _permute(text_features.T.astype(ml_dtypes.float8_e4m3), 512)
    ones8 = np.ones((128, 256), dtype=ml_dtypes.float8_e4m3)
    onesb = np.ones((128, 128), dtype=ml_dtypes.bfloat16)
    maps = []
    for m in range(n_cores):
        sl = slice(m * rows, (m + 1) * rows)
        maps.append({
            "imgT": _permute(
                image_features[sl].T.astype(ml_dtypes.bfloat16), 512),
            "totT": _permute(
                text_features[sl].T.astype(ml_dtypes.bfloat16), 512),
            "txt8": txt8,
            "ones8": ones8,
            "onesb": onesb,
        })
    return maps


_CACHE = {}
_LOCK = threading.Lock()


def _get_nc():
    with _LOCK:
        if "nc" not in _CACHE:
            _CACHE["nc"] = build_nc()
        return _CACHE["nc"]


def kernel(image_features, text_features):
    image_features = np.asarray(image_features, dtype=np.float32)
    text_features = np.asarray(text_features, dtype=np.float32)
    assert image_features.shape == (N, D) and text_features.shape == (N, D)
    nc = _get_nc()
    in_maps = make_in_maps(image_features, text_features)
    res = run_bass_kernel_spmd(nc, in_maps, list(range(N_CORES)))
    val = np.float32(res.results[0]["out"][0, 0])
    return np.array(val, dtype=np.float32)
